# revision 1
# baseline (speedup 1.0000x reference)
"""Trainium2 Bass kernel for nn_DepParser (BiLSTM-less dep parser scorer).

Pipeline (identical SPMD program on 8 cores; only the one-hot row-selector S
differs per core):
  P1  embedding gather (indirect DMA) + PE transpose -> x^T
  P2  xg = x @ W_ih^T + b  (precomputed input projections, gate-major layout)
  P3  LSTM recurrence, 512 sequential steps.  W_hh kept bf16 and used as the
      stationary matmul operand so gates land in PSUM as [128, 16]
      (partition-parallel), which keeps the per-step ACT/DVE tail cheap.
  P4  pairwise grid: A_i + B_j + b -> tanh -> dot fc2.  Row-slab per core via
      a one-hot selection matmul (all-static SPMD, no dynamic slicing).
Output: each core writes its 65-row slab of M; host concatenates and trims.
"""

import numpy as np
import ml_dtypes

import concourse.bass as bass
import concourse.bacc as bacc
import concourse.tile as tile
from concourse import mybir
from concourse.bass_utils import run_bass_kernel_spmd
from concourse.masks import make_identity

N = 512          # sequence length
NP1 = N + 1      # grid side (root prepended)
D = 256          # embed dim
H = 512          # hidden
G = 4 * H        # gates
NCORES = 8
ROWS = 65        # grid rows per core (65*8 = 520 >= 513)

FP32 = mybir.dt.float32
BF16 = mybir.dt.bfloat16
I32 = mybir.dt.int32

AF = mybir.ActivationFunctionType

_CACHE = {}


def _build_nc():
    nc = bacc.Bacc("TRN2", target_bir_lowering=False, debug=False)

    # ---- DRAM I/O -----------------------------------------------------
    w_embed = nc.dram_tensor("w_embed", [50000, D], FP32, kind="ExternalInput")
    p_embed = nc.dram_tensor("p_embed", [50, D], FP32, kind="ExternalInput")
    words128 = nc.dram_tensor("words128", [128, 4], I32, kind="ExternalInput")
    pos128 = nc.dram_tensor("pos128", [128, 4], I32, kind="ExternalInput")
    w_ihT = nc.dram_tensor("w_ihT", [2 * D, G], FP32, kind="ExternalInput")
    w_hhT = nc.dram_tensor("w_hhT", [H, G], BF16, kind="ExternalInput")
    bih128 = nc.dram_tensor("bih128", [128, 16], FP32, kind="ExternalInput")
    bhh128 = nc.dram_tensor("bhh128", [128, 16], FP32, kind="ExternalInput")
    fc1wT = nc.dram_tensor("fc1wT", [2 * H, H], BF16, kind="ExternalInput")
    fc1b128 = nc.dram_tensor("fc1b128", [128, 4], FP32, kind="ExternalInput")
    fc2wT128 = nc.dram_tensor("fc2wT128", [128, 4], FP32, kind="ExternalInput")
    fc2b11 = nc.dram_tensor("fc2b11", [1, 1], FP32, kind="ExternalInput")
    sel = nc.dram_tensor("sel", [640, ROWS], FP32, kind="ExternalInput")
    m_slab = nc.dram_tensor("m_slab", [ROWS, NP1], FP32, kind="ExternalOutput")

    with tile.TileContext(nc) as tc:
        with tc.tile_pool(name="persist", bufs=1) as persist:
            # ---- persistent SBUF tensors ------------------------------
            wih_sb = persist.tile([128, 4, G], FP32, tag="wih")
            whh_sb = persist.tile([128, 4, G], BF16, tag="whh")
            fc1w_sb = persist.tile([128, 8, H], BF16, tag="fc1w")
            bsum_sb = persist.tile([128, 16], FP32, tag="bsum")
            fc1b_sb = persist.tile([128, 4], FP32, tag="fc1b")
            vT_sb = persist.tile([128, 4], FP32, tag="vT")
            fc2b_sb = persist.tile([1, 1], FP32, tag="fc2b")
            sel_sb = persist.tile([128, 5, ROWS], FP32, tag="sel")
            houtT = persist.tile([128, 4, NP1], BF16, tag="houtT")
            xg_sb = persist.tile([128, 16, N], FP32, tag="xg")
            bt_sb = persist.tile([128, 4, NP1], FP32, tag="bt")
            at_slab = persist.tile([128, 4, ROWS], FP32, tag="atslab")
            ident = persist.tile([128, 128], FP32, tag="ident")
            c_state = persist.tile([128, 4, 2], FP32, tag="cstate")

            for dg in range(4):
                nc.sync.dma_start(out=wih_sb[:, dg, :], in_=w_ihT[128 * dg:128 * (dg + 1), :])
                nc.sync.dma_start(out=whh_sb[:, dg, :], in_=w_hhT[128 * dg:128 * (dg + 1), :])
            for c8 in range(8):
                nc.sync.dma_start(out=fc1w_sb[:, c8, :], in_=fc1wT[128 * c8:128 * (c8 + 1), :])
            for ic in range(5):
                nc.sync.dma_start(out=sel_sb[:, ic, :], in_=sel[128 * ic:128 * (ic + 1), :])
            nc.sync.dma_start(out=fc1b_sb[:], in_=fc1b128[:])
            nc.sync.dma_start(out=vT_sb[:], in_=fc2wT128[:])
            nc.sync.dma_start(out=fc2b_sb[:], in_=fc2b11[:])
            make_identity(nc, ident[:])
            nc.vector.memset(houtT[:, :, 0], 0.0)
            nc.vector.memset(c_state[:, :, 0], 0.0)

            # b_ih + b_hh on device
            with tc.tile_pool(name="btmp", bufs=1) as btmp:
                t_bih = btmp.tile([128, 16], FP32, tag="bih")
                t_bhh = btmp.tile([128, 16], FP32, tag="bhh")
                nc.sync.dma_start(out=t_bih[:], in_=bih128[:])
                nc.sync.dma_start(out=t_bhh[:], in_=bhh128[:])
                nc.vector.tensor_add(out=bsum_sb[:], in0=t_bih[:], in1=t_bhh[:])

            # ---- P1: embedding gather + transpose ---------------------
            with (
                tc.tile_pool(name="emb", bufs=1) as emb,
                tc.tile_pool(name="p1psum", bufs=4, space="PSUM") as p1psum,
            ):
                widx = emb.tile([128, 4], I32, tag="widx")
                pidx = emb.tile([128, 4], I32, tag="pidx")
                xw = emb.tile([128, 4, D], FP32, tag="xw")
                xp = emb.tile([128, 4, D], FP32, tag="xp")
                xT = emb.tile([128, 4, N], FP32, tag="xT")
                nc.sync.dma_start(out=widx[:], in_=words128[:])
                nc.sync.dma_start(out=pidx[:], in_=pos128[:])
                for q in range(4):
                    nc.gpsimd.indirect_dma_start(
                        out=xw[:, q, :], out_offset=None,
                        in_=w_embed[:],
                        in_offset=bass.IndirectOffsetOnAxis(ap=widx[:, q:q + 1], axis=0),
                    )
                    nc.gpsimd.indirect_dma_start(
                        out=xp[:, q, :], out_offset=None,
                        in_=p_embed[:],
                        in_offset=bass.IndirectOffsetOnAxis(ap=pidx[:, q:q + 1], axis=0),
                    )
                for q in range(4):
                    for db in range(4):
                        src = xw[:, q, 128 * db:128 * (db + 1)] if db < 2 \
                            else xp[:, q, 128 * (db - 2):128 * (db - 1)]
                        pt = p1psum.tile([128, 128], FP32, tag="pt")
                        nc.tensor.transpose(out=pt[:], in_=src, identity=ident[:])
                        nc.vector.tensor_copy(
                            out=xT[:, db, 128 * q:128 * (q + 1)], in_=pt[:])

                # ---- P2: xg = x @ W_ih^T + (b_ih + b_hh) --------------
                with tc.tile_pool(name="p2psum", bufs=4, space="PSUM") as p2psum:
                    for c in range(16):
                        pxg = p2psum.tile([128, N], FP32, tag="pxg")
                        for dg in range(4):
                            nc.tensor.matmul(
                                out=pxg[:],
                                lhsT=wih_sb[:, dg, 128 * c:128 * (c + 1)],
                                rhs=xT[:, dg, :],
                                start=(dg == 0), stop=(dg == 3),
                            )
                        nc.vector.tensor_scalar_add(
                            out=xg_sb[:, c, :], in0=pxg[:],
                            scalar1=bsum_sb[:, c:c + 1],
                        )

            # ---- P3: LSTM recurrence ----------------------------------
            with (
                tc.tile_pool(name="lstm", bufs=3) as lp,
                tc.tile_pool(name="lstm_ps", bufs=2, space="PSUM") as lps,
            ):
                for t in range(N):
                    pg = lps.tile([128, 16], FP32, tag="pg")
                    for n in range(16):
                        for kg in range(4):
                            nc.tensor.matmul(
                                out=pg[:, n:n + 1],
                                lhsT=whh_sb[:, kg, 128 * n:128 * (n + 1)],
                                rhs=houtT[:, kg, t:t + 1],
                                start=(kg == 0), stop=(kg == 3),
                            )
                    pre = lp.tile([128, 16], FP32, tag="pre")
                    nc.vector.tensor_add(out=pre[:], in0=pg[:], in1=xg_sb[:, :, t])
                    acts = lp.tile([128, 16], FP32, tag="acts")
                    nc.scalar.activation(acts[:, 0:8], pre[:, 0:8], AF.Sigmoid)
                    nc.scalar.activation(acts[:, 8:12], pre[:, 8:12], AF.Tanh)
                    nc.scalar.activation(acts[:, 12:16], pre[:, 12:16], AF.Sigmoid)
                    ig = lp.tile([128, 4], FP32, tag="ig")
                    fc = lp.tile([128, 4], FP32, tag="fc")
                    nc.vector.tensor_mul(out=ig[:], in0=acts[:, 0:4], in1=acts[:, 8:12])
                    nc.vector.tensor_mul(out=fc[:], in0=acts[:, 4:8],
                                         in1=c_state[:, :, t % 2])
                    nc.vector.tensor_add(out=c_state[:, :, (t + 1) % 2],
                                         in0=ig[:], in1=fc[:])
                    tanhc = lp.tile([128, 4], FP32, tag="tanhc")
                    nc.scalar.activation(tanhc[:], c_state[:, :, (t + 1) % 2], AF.Tanh)
                    nc.vector.tensor_mul(out=houtT[:, :, t + 1],
                                         in0=acts[:, 12:16], in1=tanhc[:])

            # ---- P4: pairwise grid ------------------------------------
            # B^T[a, j] (full), A slab columns via one-hot matmul, then
            # per-row tanh + dot(v).
            with (
                tc.tile_pool(name="abphase", bufs=1) as ab,
                tc.tile_pool(name="ab_ps", bufs=2, space="PSUM") as abps,
            ):
                a_nat = ab.tile([128, 5, H], FP32, tag="anat")
                for ag in range(4):
                    pb = abps.tile([128, NP1], FP32, tag="pb")
                    for dg in range(4):
                        lhs = fc1w_sb[:, 4 + dg, 128 * ag:128 * (ag + 1)]
                        nc.tensor.matmul(out=pb[:, 0:N], lhsT=lhs,
                                         rhs=houtT[:, dg, 0:N],
                                         start=(dg == 0), stop=(dg == 3))
                        nc.tensor.matmul(out=pb[:, N:NP1], lhsT=lhs,
                                         rhs=houtT[:, dg, N:NP1],
                                         start=(dg == 0), stop=(dg == 3))
                    nc.vector.tensor_copy(out=bt_sb[:, ag, :], in_=pb[:])
                for ic in range(5):
                    mi = 128 if ic < 4 else 1
                    pa = abps.tile([128, H], FP32, tag="pa")
                    for dg in range(4):
                        nc.tensor.matmul(
                            out=pa[0:mi, :],
                            lhsT=houtT[:, dg, 128 * ic:128 * ic + mi],
                            rhs=fc1w_sb[:, dg, :],
                            start=(dg == 0), stop=(dg == 3),
                        )
                    nc.vector.tensor_copy(out=a_nat[0:mi, ic, :], in_=pa[0:mi, :])
                for ag in range(4):
                    ps = abps.tile([128, ROWS], FP32, tag="ps")
                    for ic in range(4):
                        nc.tensor.matmul(out=ps[:],
                                         lhsT=a_nat[:, ic, 128 * ag:128 * (ag + 1)],
                                         rhs=sel_sb[:, ic, :],
                                         start=(ic == 0), stop=False)
                    nc.tensor.matmul(out=ps[:],
                                     lhsT=a_nat[0:1, 4, 128 * ag:128 * (ag + 1)],
                                     rhs=sel_sb[0:1, 4, :],
                                     start=False, stop=True)
                    nc.vector.tensor_scalar_add(out=at_slab[:, ag, :], in0=ps[:],
                                                scalar1=fc1b_sb[:, ag:ag + 1])

            with (
                tc.tile_pool(name="grid", bufs=3) as gp,
                tc.tile_pool(name="grid_ps", bufs=4, space="PSUM") as gps,
                tc.tile_pool(name="grid_out", bufs=4) as go,
            ):
                for ii in range(ROWS):
                    prow = gps.tile([1, NP1], FP32, tag="prow")
                    for hg in range(4):
                        th = gp.tile([128, NP1], FP32, tag="th")
                        nc.scalar.activation(th[:], bt_sb[:, hg, :], AF.Tanh,
                                             bias=at_slab[:, hg, ii:ii + 1])
                        nc.tensor.matmul(out=prow[0:1, 0:N],
                                         lhsT=vT_sb[:, hg:hg + 1], rhs=th[:, 0:N],
                                         start=(hg == 0), stop=(hg == 3))
                        nc.tensor.matmul(out=prow[0:1, N:NP1],
                                         lhsT=vT_sb[:, hg:hg + 1], rhs=th[:, N:NP1],
                                         start=(hg == 0), stop=(hg == 3))
                    mrow = go.tile([1, NP1], FP32, tag="mrow")
                    nc.vector.tensor_scalar_add(out=mrow[:], in0=prow[:],
                                                scalar1=fc2b_sb[:])
                    nc.sync.dma_start(out=m_slab[ii:ii + 1, :], in_=mrow[:])

    nc.compile()
    return nc


def _prep_inputs(inputs):
    """Host-side layout prep (transposes / reshapes / dtype casts only)."""
    f32 = np.float32
    words = np.asarray(inputs["words"]).astype(np.int32)
    pos = np.asarray(inputs["pos"]).astype(np.int32)
    base = {
        "w_embed": np.ascontiguousarray(np.asarray(inputs["w_embed"], f32)),
        "p_embed": np.ascontiguousarray(np.asarray(inputs["p_embed"], f32)),
        "words128": np.ascontiguousarray(words.reshape(4, 128).T),
        "pos128": np.ascontiguousarray(pos.reshape(4, 128).T),
        "w_ihT": np.ascontiguousarray(np.asarray(inputs["W_ih"], f32).T),
        "w_hhT": np.ascontiguousarray(
            np.asarray(inputs["W_hh"], f32).T.astype(ml_dtypes.bfloat16)),
        "bih128": np.ascontiguousarray(
            np.asarray(inputs["b_ih"], f32).reshape(16, 128).T),
        "bhh128": np.ascontiguousarray(
            np.asarray(inputs["b_hh"], f32).reshape(16, 128).T),
        "fc1wT": np.ascontiguousarray(
            np.asarray(inputs["fc1_w"], f32).T.astype(ml_dtypes.bfloat16)),
        "fc1b128": np.ascontiguousarray(
            np.asarray(inputs["fc1_b"], f32).reshape(4, 128).T),
        "fc2wT128": np.ascontiguousarray(
            np.asarray(inputs["fc2_w"], f32)[0].reshape(4, 128).T),
        "fc2b11": np.asarray(inputs["fc2_b"], f32).reshape(1, 1),
    }
    in_maps = []
    for core in range(NCORES):
        s = np.zeros((640, ROWS), f32)
        base_row = core * ROWS
        for ii in range(ROWS):
            i = base_row + ii
            if i < NP1:
                s[i, ii] = 1.0
        in_maps.append({**base, "sel": s})
    return in_maps


def kernel(**inputs) -> np.ndarray:
    if "nc" not in _CACHE:
        _CACHE["nc"] = _build_nc()
    nc = _CACHE["nc"]
    in_maps = _prep_inputs(inputs)
    res = run_bass_kernel_spmd(nc, in_maps, list(range(NCORES)))
    slabs = [np.asarray(res.results[c]["m_slab"]) for c in range(NCORES)]
    return np.concatenate(slabs, axis=0)[:NP1, :]


if __name__ == "__main__":
    rng = np.random.default_rng(0)
    fake = {
        "words": rng.integers(0, 50000, (N,)),
        "pos": rng.integers(0, 50, (N,)),
        "w_embed": rng.standard_normal((50000, D), np.float32) * 0.05,
        "p_embed": rng.standard_normal((50, D), np.float32) * 0.05,
        "W_ih": rng.standard_normal((G, 2 * D), np.float32) * 0.05,
        "W_hh": rng.standard_normal((G, H), np.float32) * 0.05,
        "b_ih": rng.standard_normal((G,), np.float32) * 0.05,
        "b_hh": rng.standard_normal((G,), np.float32) * 0.05,
        "fc1_w": rng.standard_normal((H, 2 * H), np.float32) * 0.05,
        "fc1_b": rng.standard_normal((H,), np.float32) * 0.05,
        "fc2_w": rng.standard_normal((1, H), np.float32) * 0.05,
        "fc2_b": rng.standard_normal((1,), np.float32) * 0.05,
    }
    out = kernel(**fake)
    print("out", out.shape, out.dtype, np.abs(out).max())


# revision 2
# speedup vs baseline: 1.1422x; 1.1422x over previous
"""Trainium2 Bass kernel for nn_DepParser (BiLSTM-less dep parser scorer).

Pipeline (identical SPMD program on 8 cores; only the one-hot row-selector S
differs per core):
  P1  embedding gather (indirect DMA) + PE transpose -> x^T
  P2  xg = x @ W_ih^T + b  (precomputed input projections, gate-major layout)
  P3  LSTM recurrence, 512 sequential steps.  W_hh kept bf16 and used as the
      stationary matmul operand so gates land in PSUM as [128, 16]
      (partition-parallel), which keeps the per-step ACT/DVE tail cheap.
  P4  pairwise grid: A_i + B_j + b -> tanh -> dot fc2.  Row-slab per core via
      a one-hot selection matmul (all-static SPMD, no dynamic slicing).
Output: each core writes its 65-row slab of M; host concatenates and trims.
"""

import numpy as np
import ml_dtypes

import concourse.bass as bass
import concourse.bacc as bacc
import concourse.tile as tile
from concourse import mybir
from concourse.bass_utils import run_bass_kernel_spmd
from concourse.masks import make_identity

N = 512          # sequence length
NP1 = N + 1      # grid side (root prepended)
D = 256          # embed dim
H = 512          # hidden
G = 4 * H        # gates
NCORES = 8
ROWS = 65        # grid rows per core (65*8 = 520 >= 513)

FP32 = mybir.dt.float32
BF16 = mybir.dt.bfloat16
I32 = mybir.dt.int32

AF = mybir.ActivationFunctionType

_CACHE = {}


def _build_nc():
    nc = bacc.Bacc("TRN2", target_bir_lowering=False, debug=False)

    # ---- DRAM I/O -----------------------------------------------------
    w_embed = nc.dram_tensor("w_embed", [50000, D], FP32, kind="ExternalInput")
    p_embed = nc.dram_tensor("p_embed", [50, D], FP32, kind="ExternalInput")
    words128 = nc.dram_tensor("words128", [128, 4], I32, kind="ExternalInput")
    pos128 = nc.dram_tensor("pos128", [128, 4], I32, kind="ExternalInput")
    w_ihT = nc.dram_tensor("w_ihT", [2 * D, G], FP32, kind="ExternalInput")
    w_hhT = nc.dram_tensor("w_hhT", [H, G], BF16, kind="ExternalInput")
    bih128 = nc.dram_tensor("bih128", [128, 16], FP32, kind="ExternalInput")
    bhh128 = nc.dram_tensor("bhh128", [128, 16], FP32, kind="ExternalInput")
    fc1wT = nc.dram_tensor("fc1wT", [2 * H, H], BF16, kind="ExternalInput")
    fc1b128 = nc.dram_tensor("fc1b128", [128, 4], FP32, kind="ExternalInput")
    fc2wT128 = nc.dram_tensor("fc2wT128", [128, 4], FP32, kind="ExternalInput")
    fc2b11 = nc.dram_tensor("fc2b11", [1, 1], FP32, kind="ExternalInput")
    sel = nc.dram_tensor("sel", [640, ROWS], FP32, kind="ExternalInput")
    m_slab = nc.dram_tensor("m_slab", [ROWS, NP1], FP32, kind="ExternalOutput")

    with tile.TileContext(nc) as tc:
        with tc.tile_pool(name="persist", bufs=1) as persist:
            # ---- persistent SBUF tensors ------------------------------
            wih_sb = persist.tile([128, 4, G], FP32, tag="wih")
            whh_sb = persist.tile([128, 4, G], BF16, tag="whh")
            fc1w_sb = persist.tile([128, 8, H], BF16, tag="fc1w")
            bsum_sb = persist.tile([128, 16], FP32, tag="bsum")
            fc1b_sb = persist.tile([128, 4], FP32, tag="fc1b")
            vT_sb = persist.tile([128, 4], FP32, tag="vT")
            fc2b_sb = persist.tile([1, 1], FP32, tag="fc2b")
            sel_sb = persist.tile([128, 5, ROWS], FP32, tag="sel")
            houtT = persist.tile([128, 4, NP1], BF16, tag="houtT")
            xg_sb = persist.tile([128, 16, N], FP32, tag="xg")
            bt_sb = persist.tile([128, 4, NP1], FP32, tag="bt")
            at_slab = persist.tile([128, 4, ROWS], FP32, tag="atslab")
            ident = persist.tile([128, 128], FP32, tag="ident")
            c_state = persist.tile([128, 4, 2], FP32, tag="cstate")

            for dg in range(4):
                nc.sync.dma_start(out=wih_sb[:, dg, :], in_=w_ihT[128 * dg:128 * (dg + 1), :])
                nc.sync.dma_start(out=whh_sb[:, dg, :], in_=w_hhT[128 * dg:128 * (dg + 1), :])
            for c8 in range(8):
                nc.sync.dma_start(out=fc1w_sb[:, c8, :], in_=fc1wT[128 * c8:128 * (c8 + 1), :])
            for ic in range(5):
                nc.sync.dma_start(out=sel_sb[:, ic, :], in_=sel[128 * ic:128 * (ic + 1), :])
            nc.sync.dma_start(out=fc1b_sb[:], in_=fc1b128[:])
            nc.sync.dma_start(out=vT_sb[:], in_=fc2wT128[:])
            nc.sync.dma_start(out=fc2b_sb[:], in_=fc2b11[:])
            make_identity(nc, ident[:])
            nc.vector.memset(houtT[:, :, 0], 0.0)
            nc.vector.memset(c_state[:, :, 0], 0.0)

            # b_ih + b_hh on device
            with tc.tile_pool(name="btmp", bufs=1) as btmp:
                t_bih = btmp.tile([128, 16], FP32, tag="bih")
                t_bhh = btmp.tile([128, 16], FP32, tag="bhh")
                nc.sync.dma_start(out=t_bih[:], in_=bih128[:])
                nc.sync.dma_start(out=t_bhh[:], in_=bhh128[:])
                nc.vector.tensor_add(out=bsum_sb[:], in0=t_bih[:], in1=t_bhh[:])

            # ---- P1: embedding gather + transpose ---------------------
            with (
                tc.tile_pool(name="emb", bufs=1) as emb,
                tc.tile_pool(name="p1psum", bufs=4, space="PSUM") as p1psum,
            ):
                widx = emb.tile([128, 4], I32, tag="widx")
                pidx = emb.tile([128, 4], I32, tag="pidx")
                xw = emb.tile([128, 4, D], FP32, tag="xw")
                xp = emb.tile([128, 4, D], FP32, tag="xp")
                xT = emb.tile([128, 4, N], FP32, tag="xT")
                nc.sync.dma_start(out=widx[:], in_=words128[:])
                nc.sync.dma_start(out=pidx[:], in_=pos128[:])
                for q in range(4):
                    nc.gpsimd.indirect_dma_start(
                        out=xw[:, q, :], out_offset=None,
                        in_=w_embed[:],
                        in_offset=bass.IndirectOffsetOnAxis(ap=widx[:, q:q + 1], axis=0),
                    )
                    nc.gpsimd.indirect_dma_start(
                        out=xp[:, q, :], out_offset=None,
                        in_=p_embed[:],
                        in_offset=bass.IndirectOffsetOnAxis(ap=pidx[:, q:q + 1], axis=0),
                    )
                for q in range(4):
                    for db in range(4):
                        src = xw[:, q, 128 * db:128 * (db + 1)] if db < 2 \
                            else xp[:, q, 128 * (db - 2):128 * (db - 1)]
                        pt = p1psum.tile([128, 128], FP32, tag="pt")
                        nc.tensor.transpose(out=pt[:], in_=src, identity=ident[:])
                        nc.vector.tensor_copy(
                            out=xT[:, db, 128 * q:128 * (q + 1)], in_=pt[:])

                # ---- P2: xg = x @ W_ih^T + (b_ih + b_hh) --------------
                with tc.tile_pool(name="p2psum", bufs=4, space="PSUM") as p2psum:
                    for c in range(16):
                        pxg = p2psum.tile([128, N], FP32, tag="pxg")
                        for dg in range(4):
                            nc.tensor.matmul(
                                out=pxg[:],
                                lhsT=wih_sb[:, dg, 128 * c:128 * (c + 1)],
                                rhs=xT[:, dg, :],
                                start=(dg == 0), stop=(dg == 3),
                            )
                        nc.vector.tensor_scalar_add(
                            out=xg_sb[:, c, :], in0=pxg[:],
                            scalar1=bsum_sb[:, c:c + 1],
                        )

            # ---- P3: LSTM recurrence ----------------------------------
            with (
                tc.tile_pool(name="lstm", bufs=3) as lp,
                tc.tile_pool(name="lstm_ps", bufs=2, space="PSUM") as lps,
            ):
                for t in range(N):
                    # Separate PSUM tiles per gate group -> different banks,
                    # so the DVE pre-add of an early group can run while PE
                    # still writes a later group (same-bank PE-W/DVE-R is
                    # serialized by Tile).
                    pg_if = lps.tile([128, 8], FP32, tag="pg_if")
                    pg_g = lps.tile([128, 4], FP32, tag="pg_g")
                    pg_o = lps.tile([128, 4], FP32, tag="pg_o")

                    def _mm(dst, n):
                        for kg in range(4):
                            nc.tensor.matmul(
                                out=dst,
                                lhsT=whh_sb[:, kg, 128 * n:128 * (n + 1)],
                                rhs=houtT[:, kg, t:t + 1],
                                start=(kg == 0), stop=(kg == 3),
                            )
                    for n in range(8):
                        _mm(pg_if[:, n:n + 1], n)
                    for n in range(4):
                        _mm(pg_g[:, n:n + 1], 8 + n)
                    pre = lp.tile([128, 16], FP32, tag="pre")
                    nc.vector.tensor_add(out=pre[:, 0:8], in0=pg_if[:],
                                         in1=xg_sb[:, 0:8, t])
                    acts = lp.tile([128, 16], FP32, tag="acts")
                    nc.scalar.activation(acts[:, 0:8], pre[:, 0:8], AF.Sigmoid)
                    for n in range(4):
                        _mm(pg_o[:, n:n + 1], 12 + n)
                    nc.vector.tensor_add(out=pre[:, 8:12], in0=pg_g[:],
                                         in1=xg_sb[:, 8:12, t])
                    nc.scalar.activation(acts[:, 8:12], pre[:, 8:12], AF.Tanh)
                    nc.vector.tensor_add(out=pre[:, 12:16], in0=pg_o[:],
                                         in1=xg_sb[:, 12:16, t])
                    nc.scalar.activation(acts[:, 12:16], pre[:, 12:16], AF.Sigmoid)
                    ig = lp.tile([128, 4], FP32, tag="ig")
                    fc = lp.tile([128, 4], FP32, tag="fc")
                    nc.vector.tensor_mul(out=ig[:], in0=acts[:, 0:4], in1=acts[:, 8:12])
                    nc.gpsimd.tensor_mul(out=fc[:], in0=acts[:, 4:8],
                                         in1=c_state[:, :, t % 2])
                    nc.vector.tensor_add(out=c_state[:, :, (t + 1) % 2],
                                         in0=ig[:], in1=fc[:])
                    tanhc = lp.tile([128, 4], FP32, tag="tanhc")
                    nc.scalar.activation(tanhc[:], c_state[:, :, (t + 1) % 2], AF.Tanh)
                    nc.vector.tensor_mul(out=houtT[:, :, t + 1],
                                         in0=acts[:, 12:16], in1=tanhc[:])

            # ---- P4: pairwise grid ------------------------------------
            # B^T[a, j] (full), A slab columns via one-hot matmul, then
            # per-row tanh + dot(v).
            with (
                tc.tile_pool(name="abphase", bufs=1) as ab,
                tc.tile_pool(name="ab_ps", bufs=2, space="PSUM") as abps,
            ):
                a_nat = ab.tile([128, 5, H], FP32, tag="anat")
                for ag in range(4):
                    pb = abps.tile([128, NP1], FP32, tag="pb")
                    for dg in range(4):
                        lhs = fc1w_sb[:, 4 + dg, 128 * ag:128 * (ag + 1)]
                        nc.tensor.matmul(out=pb[:, 0:N], lhsT=lhs,
                                         rhs=houtT[:, dg, 0:N],
                                         start=(dg == 0), stop=(dg == 3))
                        nc.tensor.matmul(out=pb[:, N:NP1], lhsT=lhs,
                                         rhs=houtT[:, dg, N:NP1],
                                         start=(dg == 0), stop=(dg == 3))
                    nc.vector.tensor_copy(out=bt_sb[:, ag, :], in_=pb[:])
                for ic in range(5):
                    mi = 128 if ic < 4 else 1
                    pa = abps.tile([128, H], FP32, tag="pa")
                    for dg in range(4):
                        nc.tensor.matmul(
                            out=pa[0:mi, :],
                            lhsT=houtT[:, dg, 128 * ic:128 * ic + mi],
                            rhs=fc1w_sb[:, dg, :],
                            start=(dg == 0), stop=(dg == 3),
                        )
                    nc.vector.tensor_copy(out=a_nat[0:mi, ic, :], in_=pa[0:mi, :])
                for ag in range(4):
                    ps = abps.tile([128, ROWS], FP32, tag="ps")
                    for ic in range(4):
                        nc.tensor.matmul(out=ps[:],
                                         lhsT=a_nat[:, ic, 128 * ag:128 * (ag + 1)],
                                         rhs=sel_sb[:, ic, :],
                                         start=(ic == 0), stop=False)
                    nc.tensor.matmul(out=ps[:],
                                     lhsT=a_nat[0:1, 4, 128 * ag:128 * (ag + 1)],
                                     rhs=sel_sb[0:1, 4, :],
                                     start=False, stop=True)
                    nc.vector.tensor_scalar_add(out=at_slab[:, ag, :], in0=ps[:],
                                                scalar1=fc1b_sb[:, ag:ag + 1])

            with (
                tc.tile_pool(name="grid", bufs=3) as gp,
                tc.tile_pool(name="grid_ps", bufs=4, space="PSUM") as gps,
                tc.tile_pool(name="grid_out", bufs=4) as go,
            ):
                for ii in range(ROWS):
                    prow = gps.tile([1, NP1], FP32, tag="prow")
                    for hg in range(4):
                        th = gp.tile([128, NP1], FP32, tag="th")
                        nc.scalar.activation(th[:], bt_sb[:, hg, :], AF.Tanh,
                                             bias=at_slab[:, hg, ii:ii + 1])
                        nc.tensor.matmul(out=prow[0:1, 0:N],
                                         lhsT=vT_sb[:, hg:hg + 1], rhs=th[:, 0:N],
                                         start=(hg == 0), stop=(hg == 3))
                        nc.tensor.matmul(out=prow[0:1, N:NP1],
                                         lhsT=vT_sb[:, hg:hg + 1], rhs=th[:, N:NP1],
                                         start=(hg == 0), stop=(hg == 3))
                    mrow = go.tile([1, NP1], FP32, tag="mrow")
                    nc.vector.tensor_scalar_add(out=mrow[:], in0=prow[:],
                                                scalar1=fc2b_sb[:])
                    nc.sync.dma_start(out=m_slab[ii:ii + 1, :], in_=mrow[:])

    nc.compile()
    return nc


def _prep_inputs(inputs):
    """Host-side layout prep (transposes / reshapes / dtype casts only)."""
    f32 = np.float32
    words = np.asarray(inputs["words"]).astype(np.int32)
    pos = np.asarray(inputs["pos"]).astype(np.int32)
    base = {
        "w_embed": np.ascontiguousarray(np.asarray(inputs["w_embed"], f32)),
        "p_embed": np.ascontiguousarray(np.asarray(inputs["p_embed"], f32)),
        "words128": np.ascontiguousarray(words.reshape(4, 128).T),
        "pos128": np.ascontiguousarray(pos.reshape(4, 128).T),
        "w_ihT": np.ascontiguousarray(np.asarray(inputs["W_ih"], f32).T),
        "w_hhT": np.ascontiguousarray(
            np.asarray(inputs["W_hh"], f32).T.astype(ml_dtypes.bfloat16)),
        "bih128": np.ascontiguousarray(
            np.asarray(inputs["b_ih"], f32).reshape(16, 128).T),
        "bhh128": np.ascontiguousarray(
            np.asarray(inputs["b_hh"], f32).reshape(16, 128).T),
        "fc1wT": np.ascontiguousarray(
            np.asarray(inputs["fc1_w"], f32).T.astype(ml_dtypes.bfloat16)),
        "fc1b128": np.ascontiguousarray(
            np.asarray(inputs["fc1_b"], f32).reshape(4, 128).T),
        "fc2wT128": np.ascontiguousarray(
            np.asarray(inputs["fc2_w"], f32)[0].reshape(4, 128).T),
        "fc2b11": np.asarray(inputs["fc2_b"], f32).reshape(1, 1),
    }
    in_maps = []
    for core in range(NCORES):
        s = np.zeros((640, ROWS), f32)
        base_row = core * ROWS
        for ii in range(ROWS):
            i = base_row + ii
            if i < NP1:
                s[i, ii] = 1.0
        in_maps.append({**base, "sel": s})
    return in_maps


def kernel(**inputs) -> np.ndarray:
    if "nc" not in _CACHE:
        _CACHE["nc"] = _build_nc()
    nc = _CACHE["nc"]
    in_maps = _prep_inputs(inputs)
    res = run_bass_kernel_spmd(nc, in_maps, list(range(NCORES)))
    slabs = [np.asarray(res.results[c]["m_slab"]) for c in range(NCORES)]
    return np.concatenate(slabs, axis=0)[:NP1, :]


if __name__ == "__main__":
    rng = np.random.default_rng(0)
    fake = {
        "words": rng.integers(0, 50000, (N,)),
        "pos": rng.integers(0, 50, (N,)),
        "w_embed": rng.standard_normal((50000, D), np.float32) * 0.05,
        "p_embed": rng.standard_normal((50, D), np.float32) * 0.05,
        "W_ih": rng.standard_normal((G, 2 * D), np.float32) * 0.05,
        "W_hh": rng.standard_normal((G, H), np.float32) * 0.05,
        "b_ih": rng.standard_normal((G,), np.float32) * 0.05,
        "b_hh": rng.standard_normal((G,), np.float32) * 0.05,
        "fc1_w": rng.standard_normal((H, 2 * H), np.float32) * 0.05,
        "fc1_b": rng.standard_normal((H,), np.float32) * 0.05,
        "fc2_w": rng.standard_normal((1, H), np.float32) * 0.05,
        "fc2_b": rng.standard_normal((1,), np.float32) * 0.05,
    }
    out = kernel(**fake)
    print("out", out.shape, out.dtype, np.abs(out).max())


# revision 11
# speedup vs baseline: 1.1899x; 1.0417x over previous
"""Trainium2 Bass kernel for nn_DepParser (BiLSTM-less dep parser scorer).

Pipeline (identical SPMD program on 8 cores; only the one-hot row-selector S
differs per core):
  P1  embedding gather (indirect DMA) + PE transpose -> x^T
  P2  xg = x @ W_ih^T + b  (precomputed input projections, gate-major layout)
  P3  LSTM recurrence, 512 sequential steps.  W_hh kept bf16 and used as the
      stationary matmul operand so gates land in PSUM as [128, 16]
      (partition-parallel), which keeps the per-step ACT/DVE tail cheap.
  P4  pairwise grid: A_i + B_j + b -> tanh -> dot fc2.  Row-slab per core via
      a one-hot selection matmul (all-static SPMD, no dynamic slicing).
Output: each core writes its 65-row slab of M; host concatenates and trims.
"""

import numpy as np
import ml_dtypes

import concourse.bass as bass
import concourse.bacc as bacc
import concourse.tile as tile
from concourse import mybir
from concourse.bass_utils import run_bass_kernel_spmd
from concourse.masks import make_identity

N = 512          # sequence length
NP1 = N + 1      # grid side (root prepended)
D = 256          # embed dim
H = 512          # hidden
G = 4 * H        # gates
NCORES = 8
ROWS = 65        # grid rows per core (65*8 = 520 >= 513)

FP32 = mybir.dt.float32
BF16 = mybir.dt.bfloat16
I32 = mybir.dt.int32

AF = mybir.ActivationFunctionType

_CACHE = {}


def _build_nc():
    nc = bacc.Bacc("TRN2", target_bir_lowering=False, debug=False)

    # ---- DRAM I/O -----------------------------------------------------
    w_embed = nc.dram_tensor("w_embed", [50000, D], FP32, kind="ExternalInput")
    p_embed = nc.dram_tensor("p_embed", [50, D], FP32, kind="ExternalInput")
    words128 = nc.dram_tensor("words128", [128, 4], I32, kind="ExternalInput")
    pos128 = nc.dram_tensor("pos128", [128, 4], I32, kind="ExternalInput")
    w_ihT = nc.dram_tensor("w_ihT", [2 * D, G], BF16, kind="ExternalInput")
    w_hhT = nc.dram_tensor("w_hhT", [H, G], BF16, kind="ExternalInput")
    bih128 = nc.dram_tensor("bih128", [128, 16], FP32, kind="ExternalInput")
    bhh128 = nc.dram_tensor("bhh128", [128, 16], FP32, kind="ExternalInput")
    fc1wT = nc.dram_tensor("fc1wT", [2 * H, H], BF16, kind="ExternalInput")
    fc1b128 = nc.dram_tensor("fc1b128", [128, 4], FP32, kind="ExternalInput")
    fc2wT128 = nc.dram_tensor("fc2wT128", [128, 4], BF16, kind="ExternalInput")
    fc2b11 = nc.dram_tensor("fc2b11", [1, 1], FP32, kind="ExternalInput")
    sel = nc.dram_tensor("sel", [640, ROWS], FP32, kind="ExternalInput")
    m_slab = nc.dram_tensor("m_slab", [ROWS, NP1], FP32, kind="ExternalOutput")

    with tile.TileContext(nc) as tc:
        with tc.tile_pool(name="persist", bufs=1) as persist:
            # ---- persistent SBUF tensors ------------------------------
            wih_sb = persist.tile([128, 4, G], BF16, tag="wih")
            whh_sb = persist.tile([128, 4, G], BF16, tag="whh")
            fc1w_sb = persist.tile([128, 8, H], BF16, tag="fc1w")
            bsum_sb = persist.tile([128, 16], FP32, tag="bsum")
            fc1b_sb = persist.tile([128, 4], FP32, tag="fc1b")
            vT_sb = persist.tile([128, 4], BF16, tag="vT")
            fc2b_sb = persist.tile([1, 1], FP32, tag="fc2b")
            sel_sb = persist.tile([128, 5, ROWS], FP32, tag="sel")
            houtT = persist.tile([128, 4, NP1], BF16, tag="houtT")
            xg_sb = persist.tile([128, 16, N], FP32, tag="xg")
            bt_sb = persist.tile([128, 4, NP1], BF16, tag="bt")
            at_slab = persist.tile([128, 4, ROWS], BF16, tag="atslab")
            ident = persist.tile([128, 128], FP32, tag="ident")
            c_state = persist.tile([128, 4, 2], FP32, tag="cstate")

            for dg in range(4):
                nc.sync.dma_start(out=wih_sb[:, dg, :], in_=w_ihT[128 * dg:128 * (dg + 1), :])
                nc.sync.dma_start(out=whh_sb[:, dg, :], in_=w_hhT[128 * dg:128 * (dg + 1), :])
            for c8 in range(8):
                nc.sync.dma_start(out=fc1w_sb[:, c8, :], in_=fc1wT[128 * c8:128 * (c8 + 1), :])
            for ic in range(5):
                nc.sync.dma_start(out=sel_sb[:, ic, :], in_=sel[128 * ic:128 * (ic + 1), :])
            nc.sync.dma_start(out=fc1b_sb[:], in_=fc1b128[:])
            nc.sync.dma_start(out=vT_sb[:], in_=fc2wT128[:])
            nc.sync.dma_start(out=fc2b_sb[:], in_=fc2b11[:])
            make_identity(nc, ident[:])
            nc.vector.memset(houtT[:, :, 0], 0.0)
            nc.vector.memset(c_state[:, :, 0], 0.0)

            # b_ih + b_hh on device
            with tc.tile_pool(name="btmp", bufs=1) as btmp:
                t_bih = btmp.tile([128, 16], FP32, tag="bih")
                t_bhh = btmp.tile([128, 16], FP32, tag="bhh")
                nc.sync.dma_start(out=t_bih[:], in_=bih128[:])
                nc.sync.dma_start(out=t_bhh[:], in_=bhh128[:])
                nc.vector.tensor_add(out=bsum_sb[:], in0=t_bih[:], in1=t_bhh[:])

            # ---- P1: embedding gather + transpose ---------------------
            with (
                tc.tile_pool(name="emb", bufs=1) as emb,
                tc.tile_pool(name="p1psum", bufs=4, space="PSUM") as p1psum,
            ):
                widx = emb.tile([128, 4], I32, tag="widx")
                pidx = emb.tile([128, 4], I32, tag="pidx")
                xw = emb.tile([128, 4, D], FP32, tag="xw")
                xp = emb.tile([128, 4, D], FP32, tag="xp")
                xT = emb.tile([128, 4, N], BF16, tag="xT")
                nc.sync.dma_start(out=widx[:], in_=words128[:])
                nc.sync.dma_start(out=pidx[:], in_=pos128[:])
                for q in range(4):
                    nc.gpsimd.indirect_dma_start(
                        out=xw[:, q, :], out_offset=None,
                        in_=w_embed[:],
                        in_offset=bass.IndirectOffsetOnAxis(ap=widx[:, q:q + 1], axis=0),
                    )
                    nc.gpsimd.indirect_dma_start(
                        out=xp[:, q, :], out_offset=None,
                        in_=p_embed[:],
                        in_offset=bass.IndirectOffsetOnAxis(ap=pidx[:, q:q + 1], axis=0),
                    )
                for q in range(4):
                    for db in range(4):
                        src = xw[:, q, 128 * db:128 * (db + 1)] if db < 2 \
                            else xp[:, q, 128 * (db - 2):128 * (db - 1)]
                        pt = p1psum.tile([128, 128], FP32, tag="pt")
                        nc.tensor.transpose(out=pt[:], in_=src, identity=ident[:])
                        nc.vector.tensor_copy(
                            out=xT[:, db, 128 * q:128 * (q + 1)], in_=pt[:])

                # ---- P2: xg = x @ W_ih^T + (b_ih + b_hh) --------------
                with tc.tile_pool(name="p2psum", bufs=4, space="PSUM") as p2psum:
                    for c in range(16):
                        pxg = p2psum.tile([128, N], FP32, tag="pxg")
                        for dg in range(4):
                            nc.tensor.matmul(
                                out=pxg[:],
                                lhsT=wih_sb[:, dg, 128 * c:128 * (c + 1)],
                                rhs=xT[:, dg, :],
                                start=(dg == 0), stop=(dg == 3),
                            )
                        nc.vector.tensor_scalar_add(
                            out=xg_sb[:, c, :], in0=pxg[:],
                            scalar1=bsum_sb[:, c:c + 1],
                        )

            # ---- P3: LSTM recurrence ----------------------------------
            with (
                tc.tile_pool(name="lstm", bufs=3) as lp,
                tc.tile_pool(name="lstm_ps", bufs=2, space="PSUM") as lps,
            ):
                for t in range(N):
                    # Separate PSUM tiles per gate group -> different banks,
                    # so the DVE pre-add of an early group can run while PE
                    # still writes a later group (same-bank PE-W/DVE-R is
                    # serialized by Tile).
                    pg_if = lps.tile([128, 8], FP32, tag="pg_if")
                    pg_g = lps.tile([128, 4], FP32, tag="pg_g")
                    pg_o = lps.tile([128, 4], FP32, tag="pg_o")

                    def _mm(dst, n):
                        for kg in range(4):
                            nc.tensor.matmul(
                                out=dst,
                                lhsT=whh_sb[:, kg, 128 * n:128 * (n + 1)],
                                rhs=houtT[:, kg, t:t + 1],
                                start=(kg == 0), stop=(kg == 3),
                            )
                    for n in range(8):
                        _mm(pg_if[:, n:n + 1], n)
                    for n in range(4):
                        _mm(pg_g[:, n:n + 1], 8 + n)
                    pre = lp.tile([128, 16], FP32, tag="pre")
                    nc.vector.tensor_add(out=pre[:, 0:8], in0=pg_if[:],
                                         in1=xg_sb[:, 0:8, t])
                    acts = lp.tile([128, 16], FP32, tag="acts")
                    nc.scalar.activation(acts[:, 0:8], pre[:, 0:8], AF.Sigmoid)
                    for n in range(4):
                        _mm(pg_o[:, n:n + 1], 12 + n)
                    nc.vector.tensor_add(out=pre[:, 8:12], in0=pg_g[:],
                                         in1=xg_sb[:, 8:12, t])
                    nc.scalar.activation(acts[:, 8:12], pre[:, 8:12], AF.Tanh)
                    ig = lp.tile([128, 4], FP32, tag="ig")
                    fc = lp.tile([128, 4], FP32, tag="fc")
                    nc.gpsimd.tensor_mul(out=fc[:], in0=acts[:, 4:8],
                                         in1=c_state[:, :, t % 2])
                    nc.vector.tensor_mul(out=ig[:], in0=acts[:, 0:4], in1=acts[:, 8:12])
                    nc.vector.tensor_add(out=pre[:, 12:16], in0=pg_o[:],
                                         in1=xg_sb[:, 12:16, t])
                    nc.scalar.activation(acts[:, 12:16], pre[:, 12:16], AF.Sigmoid)
                    nc.vector.tensor_add(out=c_state[:, :, (t + 1) % 2],
                                         in0=ig[:], in1=fc[:])
                    tanhc = lp.tile([128, 4], FP32, tag="tanhc")
                    nc.scalar.activation(tanhc[:], c_state[:, :, (t + 1) % 2], AF.Tanh)
                    nc.vector.tensor_mul(out=houtT[:, :, t + 1],
                                         in0=acts[:, 12:16], in1=tanhc[:])

            # ---- P4: pairwise grid ------------------------------------
            # B^T[a, j] (full), A slab columns via one-hot matmul, then
            # per-row tanh + dot(v).
            with (
                tc.tile_pool(name="abphase", bufs=1) as ab,
                tc.tile_pool(name="ab_ps", bufs=2, space="PSUM") as abps,
            ):
                a_nat = ab.tile([128, 5, H], FP32, tag="anat")
                for ag in range(4):
                    pb = abps.tile([128, NP1], FP32, tag="pb")
                    for dg in range(4):
                        lhs = fc1w_sb[:, 4 + dg, 128 * ag:128 * (ag + 1)]
                        nc.tensor.matmul(out=pb[:, 0:N], lhsT=lhs,
                                         rhs=houtT[:, dg, 0:N],
                                         start=(dg == 0), stop=(dg == 3))
                        nc.tensor.matmul(out=pb[:, N:NP1], lhsT=lhs,
                                         rhs=houtT[:, dg, N:NP1],
                                         start=(dg == 0), stop=(dg == 3))
                    nc.vector.tensor_copy(out=bt_sb[:, ag, :], in_=pb[:])
                for ic in range(5):
                    mi = 128 if ic < 4 else 1
                    pa = abps.tile([128, H], FP32, tag="pa")
                    for dg in range(4):
                        nc.tensor.matmul(
                            out=pa[0:mi, :],
                            lhsT=houtT[:, dg, 128 * ic:128 * ic + mi],
                            rhs=fc1w_sb[:, dg, :],
                            start=(dg == 0), stop=(dg == 3),
                        )
                    nc.vector.tensor_copy(out=a_nat[0:mi, ic, :], in_=pa[0:mi, :])
                for ag in range(4):
                    ps = abps.tile([128, ROWS], FP32, tag="ps")
                    for ic in range(4):
                        nc.tensor.matmul(out=ps[:],
                                         lhsT=a_nat[:, ic, 128 * ag:128 * (ag + 1)],
                                         rhs=sel_sb[:, ic, :],
                                         start=(ic == 0), stop=False)
                    nc.tensor.matmul(out=ps[:],
                                     lhsT=a_nat[0:1, 4, 128 * ag:128 * (ag + 1)],
                                     rhs=sel_sb[0:1, 4, :],
                                     start=False, stop=True)
                    nc.vector.tensor_scalar_add(out=at_slab[:, ag, :], in0=ps[:],
                                                scalar1=fc1b_sb[:, ag:ag + 1])

            with (
                tc.tile_pool(name="grid", bufs=3) as gp,
                tc.tile_pool(name="grid_ps", bufs=4, space="PSUM") as gps,
                tc.tile_pool(name="grid_out", bufs=4) as go,
            ):
                for ii in range(ROWS):
                    prow = gps.tile([1, NP1], FP32, tag="prow")
                    # pre = B^T + A'_i broadcast along j (step-0 AP), all 4
                    # h-groups in one [128, 4*NP1] bf16 op; then one big tanh.
                    pre4 = gp.tile([128, 4, NP1], BF16, tag="pre4")
                    a_col = at_slab[:, :, ii:ii + 1]
                    a_bcast = bass.AP(
                        tensor=a_col.tensor, offset=a_col.offset,
                        ap=[a_col.ap[0], a_col.ap[1], [0, NP1]])
                    nc.vector.tensor_add(out=pre4[:], in0=bt_sb[:], in1=a_bcast)
                    th = gp.tile([128, 4, NP1], BF16, tag="th")
                    nc.scalar.activation(th[:], pre4[:], AF.Tanh)
                    for hg in range(4):
                        nc.tensor.matmul(out=prow[0:1, 0:N],
                                         lhsT=vT_sb[:, hg:hg + 1],
                                         rhs=th[:, hg, 0:N],
                                         start=(hg == 0), stop=(hg == 3))
                        nc.tensor.matmul(out=prow[0:1, N:NP1],
                                         lhsT=vT_sb[:, hg:hg + 1],
                                         rhs=th[:, hg, N:NP1],
                                         start=(hg == 0), stop=(hg == 3))
                    mrow = go.tile([1, NP1], FP32, tag="mrow")
                    nc.vector.tensor_scalar_add(out=mrow[:], in0=prow[:],
                                                scalar1=fc2b_sb[:])
                    nc.sync.dma_start(out=m_slab[ii:ii + 1, :], in_=mrow[:])

    nc.compile()
    return nc


def _prep_inputs(inputs):
    """Host-side layout prep (transposes / reshapes / dtype casts only)."""
    f32 = np.float32
    words = np.asarray(inputs["words"]).astype(np.int32)
    pos = np.asarray(inputs["pos"]).astype(np.int32)
    base = {
        "w_embed": np.ascontiguousarray(np.asarray(inputs["w_embed"], f32)),
        "p_embed": np.ascontiguousarray(np.asarray(inputs["p_embed"], f32)),
        "words128": np.ascontiguousarray(words.reshape(4, 128).T),
        "pos128": np.ascontiguousarray(pos.reshape(4, 128).T),
        "w_ihT": np.ascontiguousarray(
            np.asarray(inputs["W_ih"], f32).T.astype(ml_dtypes.bfloat16)),
        "w_hhT": np.ascontiguousarray(
            np.asarray(inputs["W_hh"], f32).T.astype(ml_dtypes.bfloat16)),
        "bih128": np.ascontiguousarray(
            np.asarray(inputs["b_ih"], f32).reshape(16, 128).T),
        "bhh128": np.ascontiguousarray(
            np.asarray(inputs["b_hh"], f32).reshape(16, 128).T),
        "fc1wT": np.ascontiguousarray(
            np.asarray(inputs["fc1_w"], f32).T.astype(ml_dtypes.bfloat16)),
        "fc1b128": np.ascontiguousarray(
            np.asarray(inputs["fc1_b"], f32).reshape(4, 128).T),
        "fc2wT128": np.ascontiguousarray(
            np.asarray(inputs["fc2_w"], f32)[0].reshape(4, 128).T
            .astype(ml_dtypes.bfloat16)),
        "fc2b11": np.asarray(inputs["fc2_b"], f32).reshape(1, 1),
    }
    in_maps = []
    for core in range(NCORES):
        s = np.zeros((640, ROWS), f32)
        base_row = core * ROWS
        for ii in range(ROWS):
            i = base_row + ii
            if i < NP1:
                s[i, ii] = 1.0
        in_maps.append({**base, "sel": s})
    return in_maps


def kernel(**inputs) -> np.ndarray:
    if "nc" not in _CACHE:
        _CACHE["nc"] = _build_nc()
    nc = _CACHE["nc"]
    in_maps = _prep_inputs(inputs)
    res = run_bass_kernel_spmd(nc, in_maps, list(range(NCORES)))
    slabs = [np.asarray(res.results[c]["m_slab"]) for c in range(NCORES)]
    return np.concatenate(slabs, axis=0)[:NP1, :]


if __name__ == "__main__":
    rng = np.random.default_rng(0)
    fake = {
        "words": rng.integers(0, 50000, (N,)),
        "pos": rng.integers(0, 50, (N,)),
        "w_embed": rng.standard_normal((50000, D), np.float32) * 0.05,
        "p_embed": rng.standard_normal((50, D), np.float32) * 0.05,
        "W_ih": rng.standard_normal((G, 2 * D), np.float32) * 0.05,
        "W_hh": rng.standard_normal((G, H), np.float32) * 0.05,
        "b_ih": rng.standard_normal((G,), np.float32) * 0.05,
        "b_hh": rng.standard_normal((G,), np.float32) * 0.05,
        "fc1_w": rng.standard_normal((H, 2 * H), np.float32) * 0.05,
        "fc1_b": rng.standard_normal((H,), np.float32) * 0.05,
        "fc2_w": rng.standard_normal((1, H), np.float32) * 0.05,
        "fc2_b": rng.standard_normal((1,), np.float32) * 0.05,
    }
    out = kernel(**fake)
    print("out", out.shape, out.dtype, np.abs(out).max())


# revision 16
# speedup vs baseline: 1.2353x; 1.0382x over previous
"""Trainium2 Bass kernel for nn_DepParser (BiLSTM-less dep parser scorer).

Pipeline (identical SPMD program on 8 cores; only the one-hot row-selector S
differs per core):
  P1  embedding gather (indirect DMA) + PE transpose -> x^T
  P2  xg = x @ W_ih^T + b  (precomputed input projections, gate-major layout)
  P3  LSTM recurrence, 512 sequential steps.  W_hh kept bf16 and used as the
      stationary matmul operand so gates land in PSUM as [128, 16]
      (partition-parallel), which keeps the per-step ACT/DVE tail cheap.
  P4  pairwise grid: A_i + B_j + b -> tanh -> dot fc2.  Row-slab per core via
      a one-hot selection matmul (all-static SPMD, no dynamic slicing).
Output: each core writes its 65-row slab of M; host concatenates and trims.
"""

import numpy as np
import ml_dtypes

import concourse.bass as bass
import concourse.bacc as bacc
import concourse.tile as tile
from concourse import mybir
from concourse.bass_utils import run_bass_kernel_spmd
from concourse.masks import make_identity

N = 512          # sequence length
NP1 = N + 1      # grid side (root prepended)
D = 256          # embed dim
H = 512          # hidden
G = 4 * H        # gates
NCORES = 8
ROWS = 65        # grid rows per core (65*8 = 520 >= 513)

FP32 = mybir.dt.float32
BF16 = mybir.dt.bfloat16
I32 = mybir.dt.int32

AF = mybir.ActivationFunctionType

_CACHE = {}


def _build_nc():
    nc = bacc.Bacc("TRN2", target_bir_lowering=False, debug=False)

    # ---- DRAM I/O -----------------------------------------------------
    w_embed = nc.dram_tensor("w_embed", [50000, D], FP32, kind="ExternalInput")
    p_embed = nc.dram_tensor("p_embed", [50, D], FP32, kind="ExternalInput")
    words128 = nc.dram_tensor("words128", [128, 4], I32, kind="ExternalInput")
    pos128 = nc.dram_tensor("pos128", [128, 4], I32, kind="ExternalInput")
    w_ihT = nc.dram_tensor("w_ihT", [2 * D, G], BF16, kind="ExternalInput")
    w_hhT = nc.dram_tensor("w_hhT", [H, G], BF16, kind="ExternalInput")
    bih128 = nc.dram_tensor("bih128", [128, 16], FP32, kind="ExternalInput")
    bhh128 = nc.dram_tensor("bhh128", [128, 16], FP32, kind="ExternalInput")
    fc1wT = nc.dram_tensor("fc1wT", [2 * H, H], BF16, kind="ExternalInput")
    fc1b128 = nc.dram_tensor("fc1b128", [128, 4], FP32, kind="ExternalInput")
    fc2wT128 = nc.dram_tensor("fc2wT128", [128, 4], BF16, kind="ExternalInput")
    fc2b11 = nc.dram_tensor("fc2b11", [1, 1], FP32, kind="ExternalInput")
    sel = nc.dram_tensor("sel", [640, ROWS], FP32, kind="ExternalInput")
    m_slab = nc.dram_tensor("m_slab", [ROWS, NP1], FP32, kind="ExternalOutput")

    with tile.TileContext(nc) as tc:
        with tc.tile_pool(name="persist", bufs=1) as persist:
            # ---- persistent SBUF tensors ------------------------------
            wih_sb = persist.tile([128, 4, G], BF16, tag="wih")
            whh_sb = persist.tile([128, 4, G], BF16, tag="whh")
            fc1w_sb = persist.tile([128, 8, H], BF16, tag="fc1w")
            bsum_sb = persist.tile([128, 16], FP32, tag="bsum")
            fc1b_sb = persist.tile([128, 4], FP32, tag="fc1b")
            vT_sb = persist.tile([128, 4], BF16, tag="vT")
            fc2b_sb = persist.tile([1, 1], FP32, tag="fc2b")
            sel_sb = persist.tile([128, 5, ROWS], FP32, tag="sel")
            houtT = persist.tile([128, 4, NP1], BF16, tag="houtT")
            xg_sb = persist.tile([128, 16, N], FP32, tag="xg")
            bt_sb = persist.tile([128, 4, NP1], BF16, tag="bt")
            at_slab = persist.tile([128, 4, ROWS], FP32, tag="atslab")
            ident = persist.tile([128, 128], FP32, tag="ident")
            c_state = persist.tile([128, 4, 2], FP32, tag="cstate")

            for dg in range(4):
                nc.sync.dma_start(out=wih_sb[:, dg, :], in_=w_ihT[128 * dg:128 * (dg + 1), :])
                nc.sync.dma_start(out=whh_sb[:, dg, :], in_=w_hhT[128 * dg:128 * (dg + 1), :])
            make_identity(nc, ident[:])
            nc.vector.memset(houtT[:, :, 0], 0.0)
            nc.vector.memset(c_state[:, :, 0], 0.0)

            # b_ih + b_hh on device
            with tc.tile_pool(name="btmp", bufs=1) as btmp:
                t_bih = btmp.tile([128, 16], FP32, tag="bih")
                t_bhh = btmp.tile([128, 16], FP32, tag="bhh")
                nc.sync.dma_start(out=t_bih[:], in_=bih128[:])
                nc.sync.dma_start(out=t_bhh[:], in_=bhh128[:])
                nc.vector.tensor_add(out=bsum_sb[:], in0=t_bih[:], in1=t_bhh[:])

            # ---- P1: embedding gather + transpose ---------------------
            with (
                tc.tile_pool(name="emb", bufs=1) as emb,
                tc.tile_pool(name="p1psum", bufs=4, space="PSUM") as p1psum,
            ):
                widx = emb.tile([128, 4], I32, tag="widx")
                pidx = emb.tile([128, 4], I32, tag="pidx")
                xw = emb.tile([128, 4, D], FP32, tag="xw")
                xp = emb.tile([128, 4, D], FP32, tag="xp")
                xT = emb.tile([128, 4, N], BF16, tag="xT")
                # index loads go on gpsimd so they don't queue behind the
                # multi-MB weight DMAs on the sync queue
                nc.gpsimd.dma_start(out=widx[:], in_=words128[:])
                nc.gpsimd.dma_start(out=pidx[:], in_=pos128[:])
                for q in range(4):
                    nc.gpsimd.indirect_dma_start(
                        out=xw[:, q, :], out_offset=None,
                        in_=w_embed[:],
                        in_offset=bass.IndirectOffsetOnAxis(ap=widx[:, q:q + 1], axis=0),
                    )
                    nc.gpsimd.indirect_dma_start(
                        out=xp[:, q, :], out_offset=None,
                        in_=p_embed[:],
                        in_offset=bass.IndirectOffsetOnAxis(ap=pidx[:, q:q + 1], axis=0),
                    )
                for q in range(4):
                    for db in range(4):
                        src = xw[:, q, 128 * db:128 * (db + 1)] if db < 2 \
                            else xp[:, q, 128 * (db - 2):128 * (db - 1)]
                        pt = p1psum.tile([128, 128], FP32, tag="pt")
                        nc.tensor.transpose(out=pt[:], in_=src, identity=ident[:])
                        nc.vector.tensor_copy(
                            out=xT[:, db, 128 * q:128 * (q + 1)], in_=pt[:])

                # ---- P2: xg = x @ W_ih^T + (b_ih + b_hh) --------------
                with tc.tile_pool(name="p2psum", bufs=4, space="PSUM") as p2psum:
                    for c in range(16):
                        pxg = p2psum.tile([128, N], FP32, tag="pxg")
                        for dg in range(4):
                            nc.tensor.matmul(
                                out=pxg[:],
                                lhsT=wih_sb[:, dg, 128 * c:128 * (c + 1)],
                                rhs=xT[:, dg, :],
                                start=(dg == 0), stop=(dg == 3),
                            )
                        nc.vector.tensor_scalar_add(
                            out=xg_sb[:, c, :], in0=pxg[:],
                            scalar1=bsum_sb[:, c:c + 1],
                        )

            # ---- P3: LSTM recurrence ----------------------------------
            with (
                tc.tile_pool(name="lstm", bufs=3) as lp,
                tc.tile_pool(name="lstm_ps", bufs=2, space="PSUM") as lps,
            ):
                for t in range(N):
                    # Separate PSUM tiles per gate group -> different banks,
                    # so the DVE pre-add of an early group can run while PE
                    # still writes a later group (same-bank PE-W/DVE-R is
                    # serialized by Tile).
                    pg_if = lps.tile([128, 8], FP32, tag="pg_if")
                    pg_g = lps.tile([128, 4], FP32, tag="pg_g")
                    pg_o = lps.tile([128, 4], FP32, tag="pg_o")

                    def _mm(dst, n):
                        for kg in range(4):
                            nc.tensor.matmul(
                                out=dst,
                                lhsT=whh_sb[:, kg, 128 * n:128 * (n + 1)],
                                rhs=houtT[:, kg, t:t + 1],
                                start=(kg == 0), stop=(kg == 3),
                            )
                    for n in range(8):
                        _mm(pg_if[:, n:n + 1], n)
                    for n in range(4):
                        _mm(pg_g[:, n:n + 1], 8 + n)
                    pre = lp.tile([128, 16], FP32, tag="pre")
                    nc.vector.tensor_add(out=pre[:, 0:8], in0=pg_if[:],
                                         in1=xg_sb[:, 0:8, t])
                    acts = lp.tile([128, 16], FP32, tag="acts")
                    nc.scalar.activation(acts[:, 0:8], pre[:, 0:8], AF.Sigmoid)
                    for n in range(4):
                        _mm(pg_o[:, n:n + 1], 12 + n)
                    nc.vector.tensor_add(out=pre[:, 8:12], in0=pg_g[:],
                                         in1=xg_sb[:, 8:12, t])
                    nc.scalar.activation(acts[:, 8:12], pre[:, 8:12], AF.Tanh)
                    ig = lp.tile([128, 4], FP32, tag="ig")
                    fc = lp.tile([128, 4], FP32, tag="fc")
                    nc.gpsimd.tensor_mul(out=fc[:], in0=acts[:, 4:8],
                                         in1=c_state[:, :, t % 2])
                    nc.vector.tensor_mul(out=ig[:], in0=acts[:, 0:4], in1=acts[:, 8:12])
                    nc.vector.tensor_add(out=pre[:, 12:16], in0=pg_o[:],
                                         in1=xg_sb[:, 12:16, t])
                    nc.scalar.activation(acts[:, 12:16], pre[:, 12:16], AF.Sigmoid)
                    nc.vector.tensor_add(out=c_state[:, :, (t + 1) % 2],
                                         in0=ig[:], in1=fc[:])
                    tanhc = lp.tile([128, 4], FP32, tag="tanhc")
                    nc.scalar.activation(tanhc[:], c_state[:, :, (t + 1) % 2], AF.Tanh)
                    nc.vector.tensor_mul(out=houtT[:, :, t + 1],
                                         in0=acts[:, 12:16], in1=tanhc[:])

            # ---- P4: pairwise grid ------------------------------------
            # B^T[a, j] (full), A slab columns via one-hot matmul, then
            # per-row tanh + dot(v).
            with (
                tc.tile_pool(name="abphase", bufs=1) as ab,
                tc.tile_pool(name="ab_ps", bufs=2, space="PSUM") as abps,
            ):
                a_nat = ab.tile([128, 5, H], FP32, tag="anat")
                # grid-only constants: loaded here so the DMAs run during
                # the LSTM phase instead of delaying it
                for c8 in range(8):
                    nc.sync.dma_start(out=fc1w_sb[:, c8, :],
                                      in_=fc1wT[128 * c8:128 * (c8 + 1), :])
                for ic in range(5):
                    nc.sync.dma_start(out=sel_sb[:, ic, :],
                                      in_=sel[128 * ic:128 * (ic + 1), :])
                nc.sync.dma_start(out=fc1b_sb[:], in_=fc1b128[:])
                nc.sync.dma_start(out=vT_sb[:], in_=fc2wT128[:])
                nc.sync.dma_start(out=fc2b_sb[:], in_=fc2b11[:])
                for ag in range(4):
                    pb = abps.tile([128, NP1], FP32, tag="pb")
                    for dg in range(4):
                        lhs = fc1w_sb[:, 4 + dg, 128 * ag:128 * (ag + 1)]
                        nc.tensor.matmul(out=pb[:, 0:N], lhsT=lhs,
                                         rhs=houtT[:, dg, 0:N],
                                         start=(dg == 0), stop=(dg == 3))
                        nc.tensor.matmul(out=pb[:, N:NP1], lhsT=lhs,
                                         rhs=houtT[:, dg, N:NP1],
                                         start=(dg == 0), stop=(dg == 3))
                    nc.vector.tensor_copy(out=bt_sb[:, ag, :], in_=pb[:])
                for ic in range(5):
                    mi = 128 if ic < 4 else 1
                    pa = abps.tile([128, H], FP32, tag="pa")
                    for dg in range(4):
                        nc.tensor.matmul(
                            out=pa[0:mi, :],
                            lhsT=houtT[:, dg, 128 * ic:128 * ic + mi],
                            rhs=fc1w_sb[:, dg, :],
                            start=(dg == 0), stop=(dg == 3),
                        )
                    nc.vector.tensor_copy(out=a_nat[0:mi, ic, :], in_=pa[0:mi, :])
                for ag in range(4):
                    ps = abps.tile([128, ROWS], FP32, tag="ps")
                    for ic in range(4):
                        nc.tensor.matmul(out=ps[:],
                                         lhsT=a_nat[:, ic, 128 * ag:128 * (ag + 1)],
                                         rhs=sel_sb[:, ic, :],
                                         start=(ic == 0), stop=False)
                    nc.tensor.matmul(out=ps[:],
                                     lhsT=a_nat[0:1, 4, 128 * ag:128 * (ag + 1)],
                                     rhs=sel_sb[0:1, 4, :],
                                     start=False, stop=True)
                    nc.vector.tensor_scalar_add(out=at_slab[:, ag, :], in0=ps[:],
                                                scalar1=fc1b_sb[:, ag:ag + 1])

            with (
                tc.tile_pool(name="grid", bufs=3) as gp,
                tc.tile_pool(name="grid_ps", bufs=4, space="PSUM") as gps,
                tc.tile_pool(name="grid_out", bufs=4) as go,
            ):
                for ii in range(ROWS):
                    prow = gps.tile([1, NP1], FP32, tag="prow")
                    # pre = B^T + A'_i broadcast along j (step-0 AP), all 4
                    # h-groups in one [128, 4*NP1] bf16 op; then one big tanh.
                    pre4 = gp.tile([128, 4, NP1], BF16, tag="pre4")
                    for hg in range(4):
                        nc.vector.tensor_scalar_add(
                            out=pre4[:, hg, :], in0=bt_sb[:, hg, :],
                            scalar1=at_slab[:, hg, ii:ii + 1])
                    th = gp.tile([128, 4, NP1], BF16, tag="th")
                    nc.scalar.activation(th[:], pre4[:], AF.Tanh)
                    for hg in range(4):
                        nc.tensor.matmul(out=prow[0:1, 0:N],
                                         lhsT=vT_sb[:, hg:hg + 1],
                                         rhs=th[:, hg, 0:N],
                                         start=(hg == 0), stop=(hg == 3))
                        nc.tensor.matmul(out=prow[0:1, N:NP1],
                                         lhsT=vT_sb[:, hg:hg + 1],
                                         rhs=th[:, hg, N:NP1],
                                         start=(hg == 0), stop=(hg == 3))
                    mrow = go.tile([1, NP1], FP32, tag="mrow")
                    nc.vector.tensor_scalar_add(out=mrow[:], in0=prow[:],
                                                scalar1=fc2b_sb[:])
                    nc.sync.dma_start(out=m_slab[ii:ii + 1, :], in_=mrow[:])

    nc.compile()
    return nc


def _prep_inputs(inputs):
    """Host-side layout prep (transposes / reshapes / dtype casts only)."""
    f32 = np.float32
    words = np.asarray(inputs["words"]).astype(np.int32)
    pos = np.asarray(inputs["pos"]).astype(np.int32)
    base = {
        "w_embed": np.ascontiguousarray(np.asarray(inputs["w_embed"], f32)),
        "p_embed": np.ascontiguousarray(np.asarray(inputs["p_embed"], f32)),
        "words128": np.ascontiguousarray(words.reshape(4, 128).T),
        "pos128": np.ascontiguousarray(pos.reshape(4, 128).T),
        "w_ihT": np.ascontiguousarray(
            np.asarray(inputs["W_ih"], f32).T.astype(ml_dtypes.bfloat16)),
        "w_hhT": np.ascontiguousarray(
            np.asarray(inputs["W_hh"], f32).T.astype(ml_dtypes.bfloat16)),
        "bih128": np.ascontiguousarray(
            np.asarray(inputs["b_ih"], f32).reshape(16, 128).T),
        "bhh128": np.ascontiguousarray(
            np.asarray(inputs["b_hh"], f32).reshape(16, 128).T),
        "fc1wT": np.ascontiguousarray(
            np.asarray(inputs["fc1_w"], f32).T.astype(ml_dtypes.bfloat16)),
        "fc1b128": np.ascontiguousarray(
            np.asarray(inputs["fc1_b"], f32).reshape(4, 128).T),
        "fc2wT128": np.ascontiguousarray(
            np.asarray(inputs["fc2_w"], f32)[0].reshape(4, 128).T
            .astype(ml_dtypes.bfloat16)),
        "fc2b11": np.asarray(inputs["fc2_b"], f32).reshape(1, 1),
    }
    in_maps = []
    for core in range(NCORES):
        s = np.zeros((640, ROWS), f32)
        base_row = core * ROWS
        for ii in range(ROWS):
            i = base_row + ii
            if i < NP1:
                s[i, ii] = 1.0
        in_maps.append({**base, "sel": s})
    return in_maps


def kernel(**inputs) -> np.ndarray:
    if "nc" not in _CACHE:
        _CACHE["nc"] = _build_nc()
    nc = _CACHE["nc"]
    in_maps = _prep_inputs(inputs)
    res = run_bass_kernel_spmd(nc, in_maps, list(range(NCORES)))
    slabs = [np.asarray(res.results[c]["m_slab"]) for c in range(NCORES)]
    return np.concatenate(slabs, axis=0)[:NP1, :]


if __name__ == "__main__":
    rng = np.random.default_rng(0)
    fake = {
        "words": rng.integers(0, 50000, (N,)),
        "pos": rng.integers(0, 50, (N,)),
        "w_embed": rng.standard_normal((50000, D), np.float32) * 0.05,
        "p_embed": rng.standard_normal((50, D), np.float32) * 0.05,
        "W_ih": rng.standard_normal((G, 2 * D), np.float32) * 0.05,
        "W_hh": rng.standard_normal((G, H), np.float32) * 0.05,
        "b_ih": rng.standard_normal((G,), np.float32) * 0.05,
        "b_hh": rng.standard_normal((G,), np.float32) * 0.05,
        "fc1_w": rng.standard_normal((H, 2 * H), np.float32) * 0.05,
        "fc1_b": rng.standard_normal((H,), np.float32) * 0.05,
        "fc2_w": rng.standard_normal((1, H), np.float32) * 0.05,
        "fc2_b": rng.standard_normal((1,), np.float32) * 0.05,
    }
    out = kernel(**fake)
    print("out", out.shape, out.dtype, np.abs(out).max())


# revision 19
# speedup vs baseline: 1.2422x; 1.0055x over previous
"""Trainium2 Bass kernel for nn_DepParser (BiLSTM-less dep parser scorer).

Pipeline (identical SPMD program on 8 cores; only the one-hot row-selector S
differs per core):
  P1  embedding gather (indirect DMA) + PE transpose -> x^T
  P2  xg = x @ W_ih^T + b  (precomputed input projections, gate-major layout)
  P3  LSTM recurrence, 512 sequential steps.  W_hh kept bf16 and used as the
      stationary matmul operand so gates land in PSUM as [128, 16]
      (partition-parallel), which keeps the per-step ACT/DVE tail cheap.
  P4  pairwise grid: A_i + B_j + b -> tanh -> dot fc2.  Row-slab per core via
      a one-hot selection matmul (all-static SPMD, no dynamic slicing).
Output: each core writes its 65-row slab of M; host concatenates and trims.
"""

import numpy as np
import ml_dtypes

import concourse.bass as bass
import concourse.bacc as bacc
import concourse.tile as tile
from concourse import mybir
from concourse.bass_utils import run_bass_kernel_spmd
from concourse.masks import make_identity

N = 512          # sequence length
NP1 = N + 1      # grid side (root prepended)
D = 256          # embed dim
H = 512          # hidden
G = 4 * H        # gates
NCORES = 8
ROWS = 65        # grid rows per core (65*8 = 520 >= 513)

FP32 = mybir.dt.float32
BF16 = mybir.dt.bfloat16
I32 = mybir.dt.int32

AF = mybir.ActivationFunctionType

_CACHE = {}


def _build_nc():
    nc = bacc.Bacc("TRN2", target_bir_lowering=False, debug=False)

    # ---- DRAM I/O -----------------------------------------------------
    w_embed = nc.dram_tensor("w_embed", [50000, D], FP32, kind="ExternalInput")
    p_embed = nc.dram_tensor("p_embed", [50, D], FP32, kind="ExternalInput")
    words128 = nc.dram_tensor("words128", [128, 4], I32, kind="ExternalInput")
    pos128 = nc.dram_tensor("pos128", [128, 4], I32, kind="ExternalInput")
    w_ihT = nc.dram_tensor("w_ihT", [2 * D, G], BF16, kind="ExternalInput")
    w_hhT = nc.dram_tensor("w_hhT", [H, G], BF16, kind="ExternalInput")
    bih128 = nc.dram_tensor("bih128", [128, 16], FP32, kind="ExternalInput")
    bhh128 = nc.dram_tensor("bhh128", [128, 16], FP32, kind="ExternalInput")
    fc1wT = nc.dram_tensor("fc1wT", [2 * H, H], BF16, kind="ExternalInput")
    fc1b128 = nc.dram_tensor("fc1b128", [128, 4], FP32, kind="ExternalInput")
    fc2wT128 = nc.dram_tensor("fc2wT128", [128, 4], BF16, kind="ExternalInput")
    fc2b11 = nc.dram_tensor("fc2b11", [1, 1], FP32, kind="ExternalInput")
    sel = nc.dram_tensor("sel", [640, ROWS], FP32, kind="ExternalInput")
    m_slab = nc.dram_tensor("m_slab", [ROWS, NP1], FP32, kind="ExternalOutput")

    with tile.TileContext(nc) as tc:
        with tc.tile_pool(name="persist", bufs=1) as persist:
            # ---- persistent SBUF tensors ------------------------------
            wih_sb = persist.tile([128, 4, G], BF16, tag="wih")
            whh_sb = persist.tile([128, 4, G], BF16, tag="whh")
            fc1w_sb = persist.tile([128, 8, H], BF16, tag="fc1w")
            bsum_sb = persist.tile([128, 16], FP32, tag="bsum")
            fc1b_sb = persist.tile([128, 4], FP32, tag="fc1b")
            vT_sb = persist.tile([128, 4], BF16, tag="vT")
            fc2b_sb = persist.tile([1, 1], FP32, tag="fc2b")
            sel_sb = persist.tile([128, 5, ROWS], FP32, tag="sel")
            houtT = persist.tile([128, 4, NP1], BF16, tag="houtT")
            xg_sb = persist.tile([128, 16, N], FP32, tag="xg")
            bt_sb = persist.tile([128, 4, NP1], BF16, tag="bt")
            at_slab = persist.tile([128, 4, ROWS], FP32, tag="atslab")
            ident = persist.tile([128, 128], FP32, tag="ident")
            c_state = persist.tile([128, 4, 2], FP32, tag="cstate")
            widx = persist.tile([128, 4], I32, tag="widx")
            pidx = persist.tile([128, 4], I32, tag="pidx")
            xw = persist.tile([128, 4, D], FP32, tag="xw")
            xp = persist.tile([128, 4, D], FP32, tag="xp")
            xT = persist.tile([128, 4, N], BF16, tag="xT")

            for dg in range(4):
                nc.sync.dma_start(out=wih_sb[:, dg, :], in_=w_ihT[128 * dg:128 * (dg + 1), :])
                nc.sync.dma_start(out=whh_sb[:, dg, :], in_=w_hhT[128 * dg:128 * (dg + 1), :])
            make_identity(nc, ident[:])
            nc.vector.memset(houtT[:, :, 0], 0.0)
            nc.vector.memset(c_state[:, :, 0], 0.0)

            # b_ih + b_hh on device
            with tc.tile_pool(name="btmp", bufs=1) as btmp:
                t_bih = btmp.tile([128, 16], FP32, tag="bih")
                t_bhh = btmp.tile([128, 16], FP32, tag="bhh")
                nc.sync.dma_start(out=t_bih[:], in_=bih128[:])
                nc.sync.dma_start(out=t_bhh[:], in_=bhh128[:])
                nc.vector.tensor_add(out=bsum_sb[:], in0=t_bih[:], in1=t_bhh[:])

            # ---- P1: embedding gather + transpose ---------------------
            with (
                tc.tile_pool(name="p1psum", bufs=4, space="PSUM") as p1psum,
            ):
                # index loads go on gpsimd so they don't queue behind the
                # multi-MB weight DMAs on the sync queue
                nc.gpsimd.dma_start(out=widx[:], in_=words128[:])
                nc.gpsimd.dma_start(out=pidx[:], in_=pos128[:])
                for q in range(4):
                    nc.gpsimd.indirect_dma_start(
                        out=xw[:, q, :], out_offset=None,
                        in_=w_embed[:],
                        in_offset=bass.IndirectOffsetOnAxis(ap=widx[:, q:q + 1], axis=0),
                    )
                    nc.gpsimd.indirect_dma_start(
                        out=xp[:, q, :], out_offset=None,
                        in_=p_embed[:],
                        in_offset=bass.IndirectOffsetOnAxis(ap=pidx[:, q:q + 1], axis=0),
                    )
                for q in range(4):
                    for db in range(4):
                        src = xw[:, q, 128 * db:128 * (db + 1)] if db < 2 \
                            else xp[:, q, 128 * (db - 2):128 * (db - 1)]
                        pt = p1psum.tile([128, 128], FP32, tag="pt")
                        nc.tensor.transpose(out=pt[:], in_=src, identity=ident[:])
                        nc.vector.tensor_copy(
                            out=xT[:, db, 128 * q:128 * (q + 1)], in_=pt[:])

                # ---- P2: xg = x @ W_ih^T + (b_ih + b_hh) --------------
                with tc.tile_pool(name="p2psum", bufs=4, space="PSUM") as p2psum:
                    for c in range(16):
                        pxg = p2psum.tile([128, N], FP32, tag="pxg")
                        for dg in range(4):
                            nc.tensor.matmul(
                                out=pxg[:],
                                lhsT=wih_sb[:, dg, 128 * c:128 * (c + 1)],
                                rhs=xT[:, dg, :],
                                start=(dg == 0), stop=(dg == 3),
                            )
                        nc.vector.tensor_scalar_add(
                            out=xg_sb[:, c, :], in0=pxg[:],
                            scalar1=bsum_sb[:, c:c + 1],
                        )

            # ---- P3: LSTM recurrence ----------------------------------
            with (
                tc.tile_pool(name="lstm", bufs=3) as lp,
                tc.tile_pool(name="lstm_ps", bufs=2, space="PSUM") as lps,
            ):
                for t in range(N):
                    # Separate PSUM tiles per gate group -> different banks,
                    # so the DVE pre-add of an early group can run while PE
                    # still writes a later group (same-bank PE-W/DVE-R is
                    # serialized by Tile).
                    pg_if = lps.tile([128, 8], FP32, tag="pg_if")
                    pg_g = lps.tile([128, 4], FP32, tag="pg_g")
                    pg_o = lps.tile([128, 4], FP32, tag="pg_o")

                    def _mm(dst, n):
                        for kg in range(4):
                            nc.tensor.matmul(
                                out=dst,
                                lhsT=whh_sb[:, kg, 128 * n:128 * (n + 1)],
                                rhs=houtT[:, kg, t:t + 1],
                                start=(kg == 0), stop=(kg == 3),
                            )
                    # g group first: its tanh is the head of the serial
                    # c-chain, and PE sem-incs drain slower than MM issues,
                    # so the earlier its columns finish the earlier the
                    # chain starts.
                    for n in range(4):
                        _mm(pg_g[:, n:n + 1], 8 + n)
                    pre = lp.tile([128, 16], FP32, tag="pre")
                    acts = lp.tile([128, 16], FP32, tag="acts")
                    nc.vector.tensor_add(out=pre[:, 8:12], in0=pg_g[:],
                                         in1=xg_sb[:, 8:12, t])
                    nc.scalar.activation(acts[:, 8:12], pre[:, 8:12], AF.Tanh)
                    for n in range(8):
                        _mm(pg_if[:, n:n + 1], n)
                    nc.vector.tensor_add(out=pre[:, 0:8], in0=pg_if[:],
                                         in1=xg_sb[:, 0:8, t])
                    nc.scalar.activation(acts[:, 0:8], pre[:, 0:8], AF.Sigmoid)
                    ig = lp.tile([128, 4], FP32, tag="ig")
                    fc = lp.tile([128, 4], FP32, tag="fc")
                    nc.gpsimd.tensor_mul(out=fc[:], in0=acts[:, 4:8],
                                         in1=c_state[:, :, t % 2])
                    nc.vector.tensor_mul(out=ig[:], in0=acts[:, 0:4], in1=acts[:, 8:12])
                    for n in range(4):
                        _mm(pg_o[:, n:n + 1], 12 + n)
                    nc.vector.tensor_add(out=pre[:, 12:16], in0=pg_o[:],
                                         in1=xg_sb[:, 12:16, t])
                    nc.scalar.activation(acts[:, 12:16], pre[:, 12:16], AF.Sigmoid)
                    nc.vector.tensor_add(out=c_state[:, :, (t + 1) % 2],
                                         in0=ig[:], in1=fc[:])
                    tanhc = lp.tile([128, 4], FP32, tag="tanhc")
                    nc.scalar.activation(tanhc[:], c_state[:, :, (t + 1) % 2], AF.Tanh)
                    nc.vector.tensor_mul(out=houtT[:, :, t + 1],
                                         in0=acts[:, 12:16], in1=tanhc[:])

            # ---- P4: pairwise grid ------------------------------------
            # B^T[a, j] (full), A slab columns via one-hot matmul, then
            # per-row tanh + dot(v).
            with (
                tc.tile_pool(name="abphase", bufs=1) as ab,
                tc.tile_pool(name="ab_ps", bufs=2, space="PSUM") as abps,
            ):
                a_nat = ab.tile([128, 5, H], FP32, tag="anat")
                # grid-only constants: loaded here so the DMAs run during
                # the LSTM phase instead of delaying it
                for c8 in range(8):
                    nc.sync.dma_start(out=fc1w_sb[:, c8, :],
                                      in_=fc1wT[128 * c8:128 * (c8 + 1), :])
                for ic in range(5):
                    nc.sync.dma_start(out=sel_sb[:, ic, :],
                                      in_=sel[128 * ic:128 * (ic + 1), :])
                nc.sync.dma_start(out=fc1b_sb[:], in_=fc1b128[:])
                nc.sync.dma_start(out=vT_sb[:], in_=fc2wT128[:])
                nc.sync.dma_start(out=fc2b_sb[:], in_=fc2b11[:])
                for ag in range(4):
                    pb = abps.tile([128, NP1], FP32, tag="pb")
                    for dg in range(4):
                        lhs = fc1w_sb[:, 4 + dg, 128 * ag:128 * (ag + 1)]
                        nc.tensor.matmul(out=pb[:, 0:N], lhsT=lhs,
                                         rhs=houtT[:, dg, 0:N],
                                         start=(dg == 0), stop=(dg == 3))
                        nc.tensor.matmul(out=pb[:, N:NP1], lhsT=lhs,
                                         rhs=houtT[:, dg, N:NP1],
                                         start=(dg == 0), stop=(dg == 3))
                    nc.vector.tensor_copy(out=bt_sb[:, ag, :], in_=pb[:])
                for ic in range(5):
                    mi = 128 if ic < 4 else 1
                    pa = abps.tile([128, H], FP32, tag="pa")
                    for dg in range(4):
                        nc.tensor.matmul(
                            out=pa[0:mi, :],
                            lhsT=houtT[:, dg, 128 * ic:128 * ic + mi],
                            rhs=fc1w_sb[:, dg, :],
                            start=(dg == 0), stop=(dg == 3),
                        )
                    nc.vector.tensor_copy(out=a_nat[0:mi, ic, :], in_=pa[0:mi, :])
                for ag in range(4):
                    ps = abps.tile([128, ROWS], FP32, tag="ps")
                    for ic in range(4):
                        nc.tensor.matmul(out=ps[:],
                                         lhsT=a_nat[:, ic, 128 * ag:128 * (ag + 1)],
                                         rhs=sel_sb[:, ic, :],
                                         start=(ic == 0), stop=False)
                    nc.tensor.matmul(out=ps[:],
                                     lhsT=a_nat[0:1, 4, 128 * ag:128 * (ag + 1)],
                                     rhs=sel_sb[0:1, 4, :],
                                     start=False, stop=True)
                    nc.vector.tensor_scalar_add(out=at_slab[:, ag, :], in0=ps[:],
                                                scalar1=fc1b_sb[:, ag:ag + 1])

            with (
                tc.tile_pool(name="grid", bufs=3) as gp,
                tc.tile_pool(name="grid_ps", bufs=4, space="PSUM") as gps,
                tc.tile_pool(name="grid_out", bufs=4) as go,
            ):
                for ii in range(ROWS):
                    prow = gps.tile([1, NP1], FP32, tag="prow")
                    # pre = B^T + A'_i broadcast along j (step-0 AP), all 4
                    # h-groups in one [128, 4*NP1] bf16 op; then one big tanh.
                    pre4 = gp.tile([128, 4, NP1], BF16, tag="pre4")
                    for hg in range(4):
                        nc.vector.tensor_scalar_add(
                            out=pre4[:, hg, :], in0=bt_sb[:, hg, :],
                            scalar1=at_slab[:, hg, ii:ii + 1])
                    th = gp.tile([128, 4, NP1], BF16, tag="th")
                    nc.scalar.activation(th[:], pre4[:], AF.Tanh)
                    for hg in range(4):
                        nc.tensor.matmul(out=prow[0:1, 0:N],
                                         lhsT=vT_sb[:, hg:hg + 1],
                                         rhs=th[:, hg, 0:N],
                                         start=(hg == 0), stop=(hg == 3))
                        nc.tensor.matmul(out=prow[0:1, N:NP1],
                                         lhsT=vT_sb[:, hg:hg + 1],
                                         rhs=th[:, hg, N:NP1],
                                         start=(hg == 0), stop=(hg == 3))
                    mrow = go.tile([1, NP1], FP32, tag="mrow")
                    nc.vector.tensor_scalar_add(out=mrow[:], in0=prow[:],
                                                scalar1=fc2b_sb[:])
                    nc.sync.dma_start(out=m_slab[ii:ii + 1, :], in_=mrow[:])

    nc.compile()
    return nc


def _prep_inputs(inputs):
    """Host-side layout prep (transposes / reshapes / dtype casts only)."""
    f32 = np.float32
    words = np.asarray(inputs["words"]).astype(np.int32)
    pos = np.asarray(inputs["pos"]).astype(np.int32)
    base = {
        "w_embed": np.ascontiguousarray(np.asarray(inputs["w_embed"], f32)),
        "p_embed": np.ascontiguousarray(np.asarray(inputs["p_embed"], f32)),
        "words128": np.ascontiguousarray(words.reshape(4, 128).T),
        "pos128": np.ascontiguousarray(pos.reshape(4, 128).T),
        "w_ihT": np.ascontiguousarray(
            np.asarray(inputs["W_ih"], f32).T.astype(ml_dtypes.bfloat16)),
        "w_hhT": np.ascontiguousarray(
            np.asarray(inputs["W_hh"], f32).T.astype(ml_dtypes.bfloat16)),
        "bih128": np.ascontiguousarray(
            np.asarray(inputs["b_ih"], f32).reshape(16, 128).T),
        "bhh128": np.ascontiguousarray(
            np.asarray(inputs["b_hh"], f32).reshape(16, 128).T),
        "fc1wT": np.ascontiguousarray(
            np.asarray(inputs["fc1_w"], f32).T.astype(ml_dtypes.bfloat16)),
        "fc1b128": np.ascontiguousarray(
            np.asarray(inputs["fc1_b"], f32).reshape(4, 128).T),
        "fc2wT128": np.ascontiguousarray(
            np.asarray(inputs["fc2_w"], f32)[0].reshape(4, 128).T
            .astype(ml_dtypes.bfloat16)),
        "fc2b11": np.asarray(inputs["fc2_b"], f32).reshape(1, 1),
    }
    in_maps = []
    for core in range(NCORES):
        s = np.zeros((640, ROWS), f32)
        base_row = core * ROWS
        for ii in range(ROWS):
            i = base_row + ii
            if i < NP1:
                s[i, ii] = 1.0
        in_maps.append({**base, "sel": s})
    return in_maps


def kernel(**inputs) -> np.ndarray:
    if "nc" not in _CACHE:
        _CACHE["nc"] = _build_nc()
    nc = _CACHE["nc"]
    in_maps = _prep_inputs(inputs)
    res = run_bass_kernel_spmd(nc, in_maps, list(range(NCORES)))
    slabs = [np.asarray(res.results[c]["m_slab"]) for c in range(NCORES)]
    return np.concatenate(slabs, axis=0)[:NP1, :]


if __name__ == "__main__":
    rng = np.random.default_rng(0)
    fake = {
        "words": rng.integers(0, 50000, (N,)),
        "pos": rng.integers(0, 50, (N,)),
        "w_embed": rng.standard_normal((50000, D), np.float32) * 0.05,
        "p_embed": rng.standard_normal((50, D), np.float32) * 0.05,
        "W_ih": rng.standard_normal((G, 2 * D), np.float32) * 0.05,
        "W_hh": rng.standard_normal((G, H), np.float32) * 0.05,
        "b_ih": rng.standard_normal((G,), np.float32) * 0.05,
        "b_hh": rng.standard_normal((G,), np.float32) * 0.05,
        "fc1_w": rng.standard_normal((H, 2 * H), np.float32) * 0.05,
        "fc1_b": rng.standard_normal((H,), np.float32) * 0.05,
        "fc2_w": rng.standard_normal((1, H), np.float32) * 0.05,
        "fc2_b": rng.standard_normal((1,), np.float32) * 0.05,
    }
    out = kernel(**fake)
    print("out", out.shape, out.dtype, np.abs(out).max())
